# revision 22
# baseline (speedup 1.0000x reference)
"""Trainium2 Bass kernel for a 2-layer edge-weighted GraphSAGE network.

Strategy (8 NeuronCores, dst-sharded):
  * Host converts the edge list (src, dst, w) into the dense row-normalized
    adjacency operator A[d, s] = sum_e w_e / max(deg_d, 1), so each layer's
    weighted segment-mean becomes a dense matmul h_N = A @ h.
  * Node ids are PADDED per core to 1280 (10 k-blocks of 128), so global
    src k-blocks align exactly with rank shards.
  * The A^T stream is CHUNK-MAJOR: the local dst range is split into
    chunks (512, 512, 226); each chunk's 40 DoubleRow pairs stream as a
    contiguous group sequence, so layer-1 aggregation finishes chunk 0
    at ~1/3 of the stream instead of at the end.  Each chunk's L1 tail
    (linear+relu -> y projection -> transpose) runs immediately, and its
    y fp8 block is exchanged RIGHT AWAY via a per-chunk AllGather - three
    small pipelined collectives that hide the CC latency under the
    remaining L1 compute and the layer-2 QA/QB/QC sweeps.
  * A dependency-free warm-up collective is rung at t~0 (garbage DRAM in,
    output unused) so the CC subsystem's init/skew window overlaps the
    whole L1 phase instead of delaying the first real exchange.
  * Dummy identity matmuls at t~0 ramp the PE DVFS pstate (0.65 -> 2.4
    GHz needs ~3us of continuous execution) before the first A tiles land.
  * Layer 2 is COMMUTED with the linear: out = x@W2a + meanagg(x@W2b).
    QA/QB/QC pair sweeps (per exchange part) run chunk-outer so each
    gather's arrival unblocks its sweep while later gathers are in flight.
  * Scales folded into host-side weights: W1b /= 64 (undoes A's x64),
    W2b *= 8 (y fp8 headroom), W2a *= 512; final activation applies 1/512
    and adds b2.
"""

import os
import sys
import types

sys.path.insert(0, "/opt/trn_rl_repo")

import numpy as np

import concourse.bacc as bacc
import concourse.tile as tile
from concourse import mybir
from concourse import bass_utils
from concourse.masks import make_identity

N_NODES = 10000
N_EDGES = 640000
D_IN, D_HID, D_OUT = 128, 256, 64
N_CORES = 8
P = 128
NB = N_NODES // N_CORES          # 1250 real dst nodes per core
NBP = 1280                       # padded local nodes (10 k-blocks)
LKB = NBP // P                   # 10 local k-blocks per core
KB = N_CORES * LKB               # 80 global k-blocks
KQ = KB // 2                     # 40 DoubleRow pairs
NPAD = KB * P                    # 10240 padded global nodes
ASCALE = 64.0                    # fp8 pre-scale on A (undone in W1b / out act)
YSCALE = 8.0                     # fp8 pre-scale on y, folded into W2b
OSCALE = ASCALE * YSCALE         # 512; folded into W2a on the host
F8 = mybir.dt.float8e4
F16 = mybir.dt.float16
F32 = mybir.dt.float32

# free-axis chunks of the local dst range (PSUM bank = 512 f32)
N_CHUNKS = [(0, 512), (512, 1024), (1024, NB)]
# (pairs per A-stream group, number of groups) per chunk; DMA descriptor
# generation runs ~50ns/row per queue, so rows are kept ~9-10KB
CGRP = [(10, 4), (10, 4), (20, 2)]
# packed f16 constants: ht | W1 | W2a' | W2b' | b1 | b2  (one 4KB-row DMA)
HC_HT, HC_W1, HC_W2A, HC_W2B, HC_B1, HC_B2 = 0, 1250, 1762, 1890, 2018, 2020
HC_W = 2021
# dst 128-blocks per chunk: c0 -> blocks 0-3, c1 -> 4-7, c2 -> 8-9(98)
CHUNK_BLOCKS = [[(0, 128), (128, 256), (256, 384), (384, 512)],
                [(0, 128), (128, 256), (256, 384), (384, 512)],
                [(0, 128), (128, 226)]]
# two exchange parts: blocks 0-7 (chunks 0+1), blocks 8-9 (chunk 2)
PART_NBLK = [8, 2]               # y blocks exchanged per part
PART_PAIRS = [[0, 1, 2, 3], [4]]     # local pair index i within a rank

_compiled_nc = None
LAST_EXEC_NS = None


def _build_nc():
    nc = bacc.Bacc("TRN2", target_bir_lowering=False, debug=False,
                   num_devices=N_CORES)

    asc_d = []
    for ci, (n0, n1) in enumerate(N_CHUNKS):
        gp, ng = CGRP[ci]
        asc_d.append(nc.dram_tensor(f"asc{ci}", [ng, P, gp * 2 * (n1 - n0)],
                                    F8, kind="ExternalInput"))
    h8_d = nc.dram_tensor("h8", [P, KB * D_IN], F8, kind="ExternalInput")
    hcat_d = nc.dram_tensor("hcat", [P, HC_W], F16, kind="ExternalInput")
    out_d = nc.dram_tensor("outT", [D_OUT, NB], F32, kind="ExternalOutput")

    with tile.TileContext(nc) as tc:
        with (
            tc.tile_pool(name="const", bufs=1) as cpool,
            tc.tile_pool(name="acache", bufs=1) as acpool,
            tc.tile_pool(name="work", bufs=1) as wpool,
            tc.tile_pool(name="dram", bufs=1, space="DRAM") as dpool,
        ):
            # ---- A^T chunk-major stream + inputs.  3 queues (sync /
            # scalar / gpsimd); vector stays clean for CC triggers + tail.
            acq = [[acpool.tile([P, gp * 2 * (n1 - n0)], F8,
                                name=f"acq_{ci}_{g}") for g in range(ng)]
                   for ci, ((n0, n1), (gp, ng)) in
                   enumerate(zip(N_CHUNKS, CGRP))]
            h8 = cpool.tile([P, KB * D_IN], F8)
            hcat = cpool.tile([P, HC_W], F16)

            # h8 in thirds (one per queue, first), packed consts on scalar,
            # then the A tiles per-queue in consumption order.  Each queue
            # gets ~5MB / ~550 descriptors -> drains ~37us.
            # NOTE: the gpsimd DMA queue is software-DGE, driven by the same
            # gpsimd core complex that executes collectives; keeping the A
            # stream off it leaves the CC cores free to initialize.
            H1, H2 = 3584, 7168
            nc.sync.dma_start(out=h8[:, :H1], in_=h8_d[:, :H1])
            nc.sync.dma_start(out=h8[:, H1:H2], in_=h8_d[:, H1:H2])
            nc.scalar.dma_start(out=h8[:, H2:], in_=h8_d[:, H2:])
            nc.scalar.dma_start(out=hcat[:], in_=hcat_d[:])
            # first c0 tile split in halves across sync+scalar for fast start
            HW0 = CGRP[0][0] * 2 * 512 // 2
            nc.sync.dma_start(out=acq[0][0][:, :HW0], in_=asc_d[0][0][:, :HW0])
            nc.scalar.dma_start(out=acq[0][0][:, HW0:], in_=asc_d[0][0][:, HW0:])
            for eng, tiles in [
                (nc.sync, [(0, 2), (1, 0), (1, 3), (2, 1)]),
                (nc.scalar, [(0, 1), (0, 3), (1, 1), (1, 2), (2, 0)]),
            ]:
                for ci, g in tiles:
                    eng.dma_start(out=acq[ci][g][:], in_=asc_d[ci][g])

            hts = hcat[:, HC_HT:HC_HT + NB]
            w2as = hcat[:, HC_W2A:HC_W2A + 2 * D_OUT]
            w2bs = hcat[:, HC_W2B:HC_W2B + 2 * D_OUT]
            b1s = hcat[:, HC_B1:HC_B1 + 2]
            b2s = hcat[0:D_OUT, HC_B2:HC_B2 + 1]

            def w1v(k, m):
                o = HC_W1 + k * D_HID + m * P
                return hcat[:, o:o + P]

            ident = cpool.tile([P, P], F16)
            make_identity(nc, ident[:])
            warm_t = cpool.tile([P, 512], F16)
            nc.vector.memset(warm_t[:], 0.0)

            # ---- PE DVFS warm-up: ~10 x 512-col f16 matmuls on zeros keep
            # the Tensor engine continuously busy from t~1us so the clock
            # is ramped when the first real A tiles land.
            with tc.tile_pool(name="psw", bufs=1, space="PSUM") as psw:
                warm_ps = psw.tile([P, 512], F32, space="PSUM")
                for _ in range(6):
                    nc.tensor.matmul(out=warm_ps[:], lhsT=ident[:],
                                     rhs=warm_t[:], start=True, stop=True)

            def h_pair(q):
                return h8[:, (2 * q) * D_IN:(2 * q + 2) * D_IN] \
                    .rearrange("p (two f) -> p two f", two=2)

            def a_pair(ci, q):
                gp, _ = CGRP[ci]
                cw = N_CHUNKS[ci][1] - N_CHUNKS[ci][0]
                g, j = q // gp, q % gp
                return acq[ci][g][:, j * 2 * cw:(j + 1) * 2 * cw] \
                    .rearrange("p (two d) -> p two d", two=2)

            hNTc = [wpool.tile([P, 512], F16, name=f"hNT{ci}")
                    for ci in range(3)]
            xTc = [[wpool.tile([P, 512], F16, name=f"xT{ci}_{m}")
                    for m in range(2)] for ci in range(3)]
            yT16c = [wpool.tile([D_OUT, 512], F16, name=f"yT16{ci}")
                     for ci in range(3)]
            y8p = [wpool.tile([P, PART_NBLK[pi] * D_OUT], F8,
                              name=f"y8p{pi}") for pi in range(2)]
            yq = wpool.tile([P, KB * D_OUT], F8)
            outsb = [wpool.tile([D_OUT, 512], F32, name=f"outsb{ci}")
                     for ci in range(3)]
            # pad rows of the last y block could hold fp8 NaNs otherwise
            nc.vector.memset(y8p[1][:], 0.0)

            ag_in = [dpool.tile([P, PART_NBLK[pi] * D_OUT], F8,
                                name=f"agin{pi}") for pi in range(2)]
            ag_out = [dpool.tile([N_CORES * P, PART_NBLK[pi] * D_OUT], F8,
                                 addr_space="Shared", name=f"agout{pi}")
                      for pi in range(2)]
            # chunk -> (exchange part, column offset in the part's y tile)
            CPART = [(0, 0), (0, 4 * D_OUT), (1, 0)]

            # ---- L1, chunk-outer: agg -> tail -> exchange, per chunk -----
            with (
                tc.tile_pool(name="psh", bufs=2, space="PSUM") as psh,
                tc.tile_pool(name="psx", bufs=2, space="PSUM") as psx,
                tc.tile_pool(name="psy", bufs=2, space="PSUM") as psy,
                tc.tile_pool(name="pst", bufs=1, space="PSUM") as pst,
            ):
                for ci, (n0, n1) in enumerate(N_CHUNKS):
                    cw = n1 - n0
                    hN_ps = psh.tile([P, 512], F32, space="PSUM", name="hN")
                    for q in range(KQ):
                        nc.tensor.matmul(
                            out=hN_ps[:, :cw],
                            lhsT=h_pair(q), rhs=a_pair(ci, q),
                            perf_mode=mybir.MatmulPerfMode.DoubleRow,
                            start=(q == 0), stop=(q == KQ - 1))
                    nc.vector.tensor_copy(out=hNTc[ci][:, :cw],
                                          in_=hN_ps[:, :cw])
                    cat1 = [hts[:, n0:n1], hNTc[ci][:, :cw]]
                    for m in range(2):
                        x_ps = psx.tile([P, 512], F32, space="PSUM",
                                        name="x_ps")
                        for k in range(2):
                            nc.tensor.matmul(
                                out=x_ps[:, :cw],
                                lhsT=w1v(k, m),
                                rhs=cat1[k],
                                start=(k == 0), stop=(k == 1))
                        nc.scalar.activation(
                            out=xTc[ci][m][:, :cw], in_=x_ps[:, :cw],
                            func=mybir.ActivationFunctionType.Relu,
                            bias=b1s[:, m:m + 1])
                    yp_ps = psy.tile([D_OUT, 512], F32, space="PSUM",
                                     name="yp_ps")
                    for k in range(2):
                        nc.tensor.matmul(
                            out=yp_ps[:, :cw],
                            lhsT=w2bs[:, k * D_OUT:(k + 1) * D_OUT],
                            rhs=xTc[ci][k][:, :cw],
                            start=(k == 0), stop=(k == 1))
                    nc.vector.tensor_copy(out=yT16c[ci][:, :cw],
                                          in_=yp_ps[:, :cw])
                    pi, pofs = CPART[ci]
                    for b, (d0, d1) in enumerate(CHUNK_BLOCKS[ci]):
                        bw = d1 - d0
                        tps = pst.tile([P, D_OUT], F16, space="PSUM",
                                       name="tps")
                        nc.tensor.transpose(out=tps[:bw, :],
                                            in_=yT16c[ci][:, d0:d1],
                                            identity=ident[:D_OUT, :D_OUT])
                        nc.vector.tensor_copy(
                            out=y8p[pi][:bw, pofs + b * D_OUT:
                                         pofs + (b + 1) * D_OUT],
                            in_=tps[:bw, :])
                    # fire the part's AllGather once its last chunk's y
                    # lands; input copy + trigger ride the idle gpsimd queue
                    if ci >= 1:
                        nc.gpsimd.dma_start(out=ag_in[pi][:], in_=y8p[pi][:])
                        nc.gpsimd.collective_compute(
                            "AllGather", mybir.AluOpType.bypass,
                            replica_groups=[list(range(N_CORES))],
                            ins=[ag_in[pi].opt()], outs=[ag_out[pi].opt()])

            # ---- unpack each gathered part: yq[p, (10r+b)*64+f] ----------
            # part ci covers local blocks [ofs, ofs+nb): contiguous source
            # run per (p, r).  Emitted after all L1 engine work so the
            # semaphore waits cannot stall the tail activations.
            yqv = yq[:].rearrange("p (r f) -> p r f", r=N_CORES)
            ueng = [nc.sync, nc.scalar]
            ofs = 0
            for pi in range(2):
                pw = PART_NBLK[pi] * D_OUT
                for i, (r0, r1) in enumerate([(0, 2), (2, 4), (4, 6), (6, 8)]):
                    ueng[(i + pi) % 2].dma_start(
                        out=yqv[:, r0:r1, ofs:ofs + pw],
                        in_=ag_out[pi][r0 * P:r1 * P]
                            .rearrange("(r p) f -> p r f", p=P))
                ofs += pw

            def yq_pair(q):
                return yq[:, (2 * q) * D_OUT:(2 * q + 2) * D_OUT] \
                    .rearrange("p (two f) -> p two f", two=2)

            # ---- layer 2: out^T = (1/512)(W2a'^T x^T + sum_q y_q^T A_q) --
            # W2a first (local), then one pair-sweep per exchange part so
            # each gather unblocks its sweep while later gathers fly.
            with tc.tile_pool(name="pso", bufs=1, space="PSUM") as pso:
                o_ps = [pso.tile([D_OUT, 512], F32, space="PSUM",
                                 name=f"o_ps{ci}") for ci in range(3)]
                for ci, (n0, n1) in enumerate(N_CHUNKS):
                    cw = n1 - n0
                    for k in range(2):
                        nc.tensor.matmul(
                            out=o_ps[ci][:, :cw],
                            lhsT=w2as[:, k * D_OUT:(k + 1) * D_OUT],
                            rhs=xTc[ci][k][:, :cw],
                            start=(k == 0), stop=False)
                for part in range(2):
                    qs = [5 * r + i for r in range(N_CORES)
                          for i in PART_PAIRS[part]]
                    last = part == 1
                    for ci, (n0, n1) in enumerate(N_CHUNKS):
                        cw = n1 - n0
                        for qi, q in enumerate(qs):
                            nc.tensor.matmul(
                                out=o_ps[ci][:, :cw],
                                lhsT=yq_pair(q), rhs=a_pair(ci, q),
                                perf_mode=mybir.MatmulPerfMode.DoubleRow,
                                start=False,
                                stop=(last and qi == len(qs) - 1))
                        if last:
                            nc.scalar.activation(
                                out=outsb[ci][:, :cw], in_=o_ps[ci][:, :cw],
                                func=mybir.ActivationFunctionType.Identity,
                                scale=1.0 / OSCALE, bias=b2s[:, 0:1])
                            nc.sync.dma_start(out=out_d[:, n0:n1],
                                              in_=outsb[ci][:, :cw])

    nc.compile()
    return nc


def _get_nc():
    global _compiled_nc
    if _compiled_nc is None:
        _compiled_nc = _build_nc()
    return _compiled_nc


def _enable_profile_hook():
    """Register the NTFF profiling hook that trn_boot skips when the image's
    antenv lacks axon_hooks (profiling only; used when GNN_PROFILE=1)."""
    try:
        import antenv
        if "antenv.axon_hooks" not in sys.modules:
            mod = types.ModuleType("antenv.axon_hooks")
            _h = [None]
            mod.set_axon_ntff_profile_hook = lambda hook: _h.__setitem__(0, hook)
            mod.get_axon_ntff_profile_hook = lambda: _h[0]
            sys.modules["antenv.axon_hooks"] = mod
            antenv.axon_hooks = mod
        from trn_agent_boot.trn_boot import _ntff_profile_via_ctypes
        hook = _ntff_profile_via_ctypes("/opt/axon/libaxon_pjrt.so")
        if hook is not None:
            sys.modules["antenv.axon_hooks"].set_axon_ntff_profile_hook(hook)
            return True
    except Exception:
        pass
    return False


def _host_prep(h, w, src, dst, W1, b1, W2, b2):
    import ml_dtypes
    import scipy.sparse as sp
    deg = np.bincount(dst, minlength=N_NODES).astype(np.float32)
    w_norm = (w[:, 0] * (ASCALE / np.maximum(deg, 1.0)[dst])).astype(np.float32)
    # pad node ids per-core to 1280 so global k-blocks align with ranks
    src_pad = (NBP * (src // NB) + src % NB).astype(np.int64)
    # AT[s_pad, d] = sum of scaled w_norm over edges (s -> d): 64*A^T
    AT = sp.coo_matrix((w_norm, (src_pad, dst)),
                       shape=(NPAD, N_NODES)).toarray()
    AT8 = AT.astype(ml_dtypes.float8_e4m3)
    hp_f = np.zeros((NPAD, D_IN), dtype=np.float32)
    for c in range(N_CORES):
        hp_f[c * NBP:c * NBP + NB] = h[c * NB:(c + 1) * NB]
    hp = hp_f.astype(ml_dtypes.float8_e4m3)
    # h8[p, k*128+f] = h[pad node 128k+p, f]
    h8 = np.ascontiguousarray(
        hp.reshape(KB, P, D_IN).transpose(1, 0, 2).reshape(P, KB * D_IN))

    # packed f16 consts: ht | W1 (hN half pre-divided by ASCALE) | W2a' |
    # W2b' | b1 | b2 -- one wide-row DMA instead of ~900 tiny descriptors
    hcat = np.zeros((P, HC_W), dtype=np.float16)
    w1c = W1.astype(np.float16)
    w1c[D_IN:] = (W1[D_IN:] / ASCALE).astype(np.float16)
    for k in range(2):
        hcat[:, HC_W1 + k * D_HID:HC_W1 + (k + 1) * D_HID] = \
            w1c[k * P:(k + 1) * P, :]
        hcat[:, HC_W2A + k * D_OUT:HC_W2A + (k + 1) * D_OUT] = (
            W2[k * P:(k + 1) * P, :] * OSCALE).astype(np.float16)
        hcat[:, HC_W2B + k * D_OUT:HC_W2B + (k + 1) * D_OUT] = (
            W2[2 * P + k * P:2 * P + (k + 1) * P, :] * YSCALE).astype(np.float16)
    hcat[:, HC_B1:HC_B1 + 2] = b1.reshape(2, P).T.astype(np.float16)
    hcat[:D_OUT, HC_B2] = b2.astype(np.float16)

    in_maps = []
    for c in range(N_CORES):
        sl = slice(c * NB, (c + 1) * NB)
        ATc = AT8[:, sl]
        hc = hcat.copy()
        hc[:, HC_HT:HC_HT + NB] = h[sl].T.astype(np.float16)
        im = {
            "h8": h8,
            "hcat": hc,
        }
        # chunk-major A stream: asc{ci}[g, p, (j two d)] for pair q=g*gp+j
        for ci, ((n0, n1), (gp, ng)) in enumerate(zip(N_CHUNKS, CGRP)):
            cw = n1 - n0
            ATcc = ATc[:, n0:n1]
            im[f"asc{ci}"] = np.ascontiguousarray(
                ATcc.reshape(ng, gp, 2, P, cw).transpose(0, 3, 1, 2, 4)
                .reshape(ng, P, gp * 2 * cw))
        in_maps.append(im)
    return in_maps


def kernel(h, w, src, dst, W1, b1, W2, b2):
    global LAST_EXEC_NS
    h = np.asarray(h, dtype=np.float32)
    w = np.asarray(w, dtype=np.float32)
    src = np.asarray(src)
    dst = np.asarray(dst)
    W1 = np.asarray(W1, dtype=np.float32)
    b1 = np.asarray(b1, dtype=np.float32)
    W2 = np.asarray(W2, dtype=np.float32)
    b2 = np.asarray(b2, dtype=np.float32)

    in_maps = _host_prep(h, w, src, dst, W1, b1, W2, b2)
    nc = _get_nc()
    trace = os.environ.get("GNN_PROFILE") == "1" and _enable_profile_hook()
    if trace:
        # steady-state measurement: first execution warms the device
        # (NEFF load, CC ring init); the traced run reflects the kernel
        bass_utils.run_bass_kernel_spmd(
            nc, in_maps, core_ids=list(range(N_CORES)), trace=False)
    res = bass_utils.run_bass_kernel_spmd(
        nc, in_maps, core_ids=list(range(N_CORES)), trace=trace)
    LAST_EXEC_NS = res.exec_time_ns

    out = np.concatenate(
        [res.results[c]["outT"].T for c in range(N_CORES)], axis=0)
    return out.astype(np.float32)


# revision 24
# speedup vs baseline: 1.0747x; 1.0747x over previous
"""Trainium2 Bass kernel for a 2-layer edge-weighted GraphSAGE network.

Strategy (8 NeuronCores, dst-sharded):
  * Host converts the edge list (src, dst, w) into the dense row-normalized
    adjacency operator A[d, s] = sum_e w_e / max(deg_d, 1), so each layer's
    weighted segment-mean becomes a dense matmul h_N = A @ h.
  * Node ids are PADDED per core to 1280 (10 k-blocks of 128), so global
    src k-blocks align exactly with rank shards.
  * The A^T stream is CHUNK-MAJOR: the local dst range is split into
    chunks (512, 512, 226); each chunk's 40 DoubleRow pairs stream as a
    contiguous group sequence, so layer-1 aggregation finishes chunk 0
    at ~1/3 of the stream instead of at the end.  Each chunk's L1 tail
    (linear+relu -> y projection -> transpose) runs immediately, and its
    y fp8 block is exchanged RIGHT AWAY via a per-chunk AllGather - three
    small pipelined collectives that hide the CC latency under the
    remaining L1 compute and the layer-2 QA/QB/QC sweeps.
  * A dependency-free warm-up collective is rung at t~0 (garbage DRAM in,
    output unused) so the CC subsystem's init/skew window overlaps the
    whole L1 phase instead of delaying the first real exchange.
  * Dummy identity matmuls at t~0 ramp the PE DVFS pstate (0.65 -> 2.4
    GHz needs ~3us of continuous execution) before the first A tiles land.
  * Layer 2 is COMMUTED with the linear: out = x@W2a + meanagg(x@W2b).
    QA/QB/QC pair sweeps (per exchange part) run chunk-outer so each
    gather's arrival unblocks its sweep while later gathers are in flight.
  * Scales folded into host-side weights: W1b /= 64 (undoes A's x64),
    W2b *= 8 (y fp8 headroom), W2a *= 512; final activation applies 1/512
    and adds b2.
"""

import os
import sys
import types

sys.path.insert(0, "/opt/trn_rl_repo")

import numpy as np

import concourse.bacc as bacc
import concourse.tile as tile
from concourse import mybir
from concourse import bass_utils
from concourse.masks import make_identity

N_NODES = 10000
N_EDGES = 640000
D_IN, D_HID, D_OUT = 128, 256, 64
N_CORES = 8
P = 128
NB = N_NODES // N_CORES          # 1250 real dst nodes per core
NBP = 1280                       # padded local nodes (10 k-blocks)
LKB = NBP // P                   # 10 local k-blocks per core
KB = N_CORES * LKB               # 80 global k-blocks
KQ = KB // 2                     # 40 DoubleRow pairs
NPAD = KB * P                    # 10240 padded global nodes
ASCALE = 64.0                    # fp8 pre-scale on A (undone in W1b / out act)
YSCALE = 8.0                     # fp8 pre-scale on y, folded into W2b
OSCALE = ASCALE * YSCALE         # 512; folded into W2a on the host
F8 = mybir.dt.float8e4
F16 = mybir.dt.float16
F32 = mybir.dt.float32

# free-axis chunks of the local dst range (PSUM bank = 512 f32)
N_CHUNKS = [(0, 512), (512, 1024), (1024, NB)]
# (pairs per A-stream group, number of groups) per chunk; DMA descriptor
# generation runs ~50ns/row per queue, so rows are kept ~9-10KB
CGRP = [(10, 4), (10, 4), (20, 2)]
# packed f16 constants: ht | W1 | W2a' | W2b' | b1 | b2  (one 4KB-row DMA)
HC_HT, HC_W1, HC_W2A, HC_W2B, HC_B1, HC_B2 = 0, 1250, 1762, 1890, 2018, 2020
HC_W = 2021
# dst 128-blocks per chunk: c0 -> blocks 0-3, c1 -> 4-7, c2 -> 8-9(98)
CHUNK_BLOCKS = [[(0, 128), (128, 256), (256, 384), (384, 512)],
                [(0, 128), (128, 256), (256, 384), (384, 512)],
                [(0, 128), (128, 226)]]
# two exchange parts: blocks 0-7 (chunks 0+1), blocks 8-9 (chunk 2)
PART_NBLK = [8, 2]               # y blocks exchanged per part
PART_PAIRS = [[0, 1, 2, 3], [4]]     # local pair index i within a rank

_compiled_nc = None
LAST_EXEC_NS = None


def _build_nc():
    nc = bacc.Bacc("TRN2", target_bir_lowering=False, debug=False,
                   num_devices=N_CORES)

    asc_d = []
    for ci, (n0, n1) in enumerate(N_CHUNKS):
        gp, ng = CGRP[ci]
        asc_d.append(nc.dram_tensor(f"asc{ci}", [ng, P, gp * 2 * (n1 - n0)],
                                    F8, kind="ExternalInput"))
    h8_d = nc.dram_tensor("h8", [P, KB * D_IN], F8, kind="ExternalInput")
    hcat_d = nc.dram_tensor("hcat", [P, HC_W], F16, kind="ExternalInput")
    out_d = nc.dram_tensor("outT", [D_OUT, NB], F32, kind="ExternalOutput")

    with tile.TileContext(nc) as tc:
        with (
            tc.tile_pool(name="const", bufs=1) as cpool,
            tc.tile_pool(name="acache", bufs=1) as acpool,
            tc.tile_pool(name="work", bufs=1) as wpool,
            tc.tile_pool(name="dram", bufs=1, space="DRAM") as dpool,
        ):
            # ---- dependency-free warm-up collective, rung at t~0: the CC
            # subsystem's ~50us init only completes promptly with an op in
            # flight, and the first op pays extra cost -- let a dummy one
            # absorb both while L1 computes.  Input is garbage DRAM; only
            # the sync effect matters.
            warm_in = dpool.tile([1, 16], F16)
            warm_out = dpool.tile([N_CORES, 16], F16, addr_space="Shared")
            nc.gpsimd.collective_compute(
                "AllGather", mybir.AluOpType.bypass,
                replica_groups=[list(range(N_CORES))],
                ins=[warm_in.opt()], outs=[warm_out.opt()])

            # ---- A^T chunk-major stream + inputs.  3 queues (sync /
            # scalar / gpsimd); vector stays clean for CC triggers + tail.
            acq = [[acpool.tile([P, gp * 2 * (n1 - n0)], F8,
                                name=f"acq_{ci}_{g}") for g in range(ng)]
                   for ci, ((n0, n1), (gp, ng)) in
                   enumerate(zip(N_CHUNKS, CGRP))]
            h8 = cpool.tile([P, KB * D_IN], F8)
            hcat = cpool.tile([P, HC_W], F16)

            # h8 in thirds (one per queue, first), packed consts on scalar,
            # then the A tiles per-queue in consumption order.  Each queue
            # gets ~5MB / ~550 descriptors -> drains ~37us.
            # NOTE: the gpsimd DMA queue is software-DGE, driven by the same
            # gpsimd core complex that executes collectives; keeping the A
            # stream off it leaves the CC cores free to initialize.
            H1, H2 = 3584, 7168
            nc.sync.dma_start(out=h8[:, :H1], in_=h8_d[:, :H1])
            nc.sync.dma_start(out=h8[:, H1:H2], in_=h8_d[:, H1:H2])
            nc.scalar.dma_start(out=h8[:, H2:], in_=h8_d[:, H2:])
            nc.scalar.dma_start(out=hcat[:], in_=hcat_d[:])
            # first c0 tile split in halves across sync+scalar for fast start
            HW0 = CGRP[0][0] * 2 * 512 // 2
            nc.sync.dma_start(out=acq[0][0][:, :HW0], in_=asc_d[0][0][:, :HW0])
            nc.scalar.dma_start(out=acq[0][0][:, HW0:], in_=asc_d[0][0][:, HW0:])
            for eng, tiles in [
                (nc.sync, [(0, 2), (1, 0), (1, 3), (2, 1)]),
                (nc.scalar, [(0, 1), (0, 3), (1, 1), (1, 2), (2, 0)]),
            ]:
                for ci, g in tiles:
                    eng.dma_start(out=acq[ci][g][:], in_=asc_d[ci][g])

            hts = hcat[:, HC_HT:HC_HT + NB]
            w2as = hcat[:, HC_W2A:HC_W2A + 2 * D_OUT]
            w2bs = hcat[:, HC_W2B:HC_W2B + 2 * D_OUT]
            b1s = hcat[:, HC_B1:HC_B1 + 2]
            b2s = hcat[0:D_OUT, HC_B2:HC_B2 + 1]

            def w1v(k, m):
                o = HC_W1 + k * D_HID + m * P
                return hcat[:, o:o + P]

            ident = cpool.tile([P, P], F16)
            make_identity(nc, ident[:])
            warm_t = cpool.tile([P, 512], F16)
            nc.vector.memset(warm_t[:], 0.0)

            # ---- PE DVFS warm-up: ~10 x 512-col f16 matmuls on zeros keep
            # the Tensor engine continuously busy from t~1us so the clock
            # is ramped when the first real A tiles land.
            with tc.tile_pool(name="psw", bufs=1, space="PSUM") as psw:
                warm_ps = psw.tile([P, 512], F32, space="PSUM")
                for _ in range(6):
                    nc.tensor.matmul(out=warm_ps[:], lhsT=ident[:],
                                     rhs=warm_t[:], start=True, stop=True)

            def h_pair(q):
                return h8[:, (2 * q) * D_IN:(2 * q + 2) * D_IN] \
                    .rearrange("p (two f) -> p two f", two=2)

            def a_pair(ci, q):
                gp, _ = CGRP[ci]
                cw = N_CHUNKS[ci][1] - N_CHUNKS[ci][0]
                g, j = q // gp, q % gp
                return acq[ci][g][:, j * 2 * cw:(j + 1) * 2 * cw] \
                    .rearrange("p (two d) -> p two d", two=2)

            hNTc = [wpool.tile([P, 512], F16, name=f"hNT{ci}")
                    for ci in range(3)]
            xTc = [[wpool.tile([P, 512], F16, name=f"xT{ci}_{m}")
                    for m in range(2)] for ci in range(3)]
            yT16c = [wpool.tile([D_OUT, 512], F16, name=f"yT16{ci}")
                     for ci in range(3)]
            y8p = [wpool.tile([P, PART_NBLK[pi] * D_OUT], F8,
                              name=f"y8p{pi}") for pi in range(2)]
            yq = wpool.tile([P, KB * D_OUT], F8)
            outsb = [wpool.tile([D_OUT, 512], F32, name=f"outsb{ci}")
                     for ci in range(3)]
            # pad rows of the last y block could hold fp8 NaNs otherwise
            nc.vector.memset(y8p[1][:], 0.0)

            ag_in = [dpool.tile([P, PART_NBLK[pi] * D_OUT], F8,
                                name=f"agin{pi}") for pi in range(2)]
            ag_out = [dpool.tile([N_CORES * P, PART_NBLK[pi] * D_OUT], F8,
                                 addr_space="Shared", name=f"agout{pi}")
                      for pi in range(2)]
            # chunk -> (exchange part, column offset in the part's y tile)
            CPART = [(0, 0), (0, 4 * D_OUT), (1, 0)]

            # ---- L1, chunk-outer: agg -> tail -> exchange, per chunk -----
            with (
                tc.tile_pool(name="psh", bufs=2, space="PSUM") as psh,
                tc.tile_pool(name="psx", bufs=2, space="PSUM") as psx,
                tc.tile_pool(name="psy", bufs=2, space="PSUM") as psy,
                tc.tile_pool(name="pst", bufs=1, space="PSUM") as pst,
            ):
                for ci, (n0, n1) in enumerate(N_CHUNKS):
                    cw = n1 - n0
                    hN_ps = psh.tile([P, 512], F32, space="PSUM", name="hN")
                    for q in range(KQ):
                        nc.tensor.matmul(
                            out=hN_ps[:, :cw],
                            lhsT=h_pair(q), rhs=a_pair(ci, q),
                            perf_mode=mybir.MatmulPerfMode.DoubleRow,
                            start=(q == 0), stop=(q == KQ - 1))
                    nc.vector.tensor_copy(out=hNTc[ci][:, :cw],
                                          in_=hN_ps[:, :cw])
                    cat1 = [hts[:, n0:n1], hNTc[ci][:, :cw]]
                    for m in range(2):
                        x_ps = psx.tile([P, 512], F32, space="PSUM",
                                        name="x_ps")
                        for k in range(2):
                            nc.tensor.matmul(
                                out=x_ps[:, :cw],
                                lhsT=w1v(k, m),
                                rhs=cat1[k],
                                start=(k == 0), stop=(k == 1))
                        nc.scalar.activation(
                            out=xTc[ci][m][:, :cw], in_=x_ps[:, :cw],
                            func=mybir.ActivationFunctionType.Relu,
                            bias=b1s[:, m:m + 1])
                    yp_ps = psy.tile([D_OUT, 512], F32, space="PSUM",
                                     name="yp_ps")
                    for k in range(2):
                        nc.tensor.matmul(
                            out=yp_ps[:, :cw],
                            lhsT=w2bs[:, k * D_OUT:(k + 1) * D_OUT],
                            rhs=xTc[ci][k][:, :cw],
                            start=(k == 0), stop=(k == 1))
                    nc.vector.tensor_copy(out=yT16c[ci][:, :cw],
                                          in_=yp_ps[:, :cw])
                    pi, pofs = CPART[ci]
                    for b, (d0, d1) in enumerate(CHUNK_BLOCKS[ci]):
                        bw = d1 - d0
                        tps = pst.tile([P, D_OUT], F16, space="PSUM",
                                       name="tps")
                        nc.tensor.transpose(out=tps[:bw, :],
                                            in_=yT16c[ci][:, d0:d1],
                                            identity=ident[:D_OUT, :D_OUT])
                        nc.vector.tensor_copy(
                            out=y8p[pi][:bw, pofs + b * D_OUT:
                                         pofs + (b + 1) * D_OUT],
                            in_=tps[:bw, :])
                    # fire the part's AllGather once its last chunk's y
                    # lands; input copy + trigger ride the idle gpsimd queue
                    if ci >= 1:
                        nc.gpsimd.dma_start(out=ag_in[pi][:], in_=y8p[pi][:])
                        nc.gpsimd.collective_compute(
                            "AllGather", mybir.AluOpType.bypass,
                            replica_groups=[list(range(N_CORES))],
                            ins=[ag_in[pi].opt()], outs=[ag_out[pi].opt()])

            # ---- unpack each gathered part: yq[p, (10r+b)*64+f] ----------
            # part ci covers local blocks [ofs, ofs+nb): contiguous source
            # run per (p, r).  Emitted after all L1 engine work so the
            # semaphore waits cannot stall the tail activations.
            yqv = yq[:].rearrange("p (r f) -> p r f", r=N_CORES)
            ueng = [nc.sync, nc.scalar]
            ofs = 0
            for pi in range(2):
                pw = PART_NBLK[pi] * D_OUT
                for i, (r0, r1) in enumerate([(0, 2), (2, 4), (4, 6), (6, 8)]):
                    ueng[(i + pi) % 2].dma_start(
                        out=yqv[:, r0:r1, ofs:ofs + pw],
                        in_=ag_out[pi][r0 * P:r1 * P]
                            .rearrange("(r p) f -> p r f", p=P))
                ofs += pw

            def yq_pair(q):
                return yq[:, (2 * q) * D_OUT:(2 * q + 2) * D_OUT] \
                    .rearrange("p (two f) -> p two f", two=2)

            # ---- layer 2: out^T = (1/512)(W2a'^T x^T + sum_q y_q^T A_q) --
            # W2a first (local), then one pair-sweep per exchange part so
            # each gather unblocks its sweep while later gathers fly.
            with tc.tile_pool(name="pso", bufs=1, space="PSUM") as pso:
                # keeper matmuls: span the CC wait so the PE DVFS clock is
                # still ramped when the gathered y lands
                keep_ps = pso.tile([P, 512], F32, space="PSUM", name="keep")
                for _ in range(35):
                    nc.tensor.matmul(out=keep_ps[:], lhsT=ident[:],
                                     rhs=warm_t[:], start=True, stop=True)
                o_ps = [pso.tile([D_OUT, 512], F32, space="PSUM",
                                 name=f"o_ps{ci}") for ci in range(3)]
                for ci, (n0, n1) in enumerate(N_CHUNKS):
                    cw = n1 - n0
                    for k in range(2):
                        nc.tensor.matmul(
                            out=o_ps[ci][:, :cw],
                            lhsT=w2as[:, k * D_OUT:(k + 1) * D_OUT],
                            rhs=xTc[ci][k][:, :cw],
                            start=(k == 0), stop=False)
                # q-outer: one weight load per pair feeds all three chunks
                for part in range(2):
                    qs = [5 * r + i for r in range(N_CORES)
                          for i in PART_PAIRS[part]]
                    last = part == 1
                    for qi, q in enumerate(qs):
                        for ci, (n0, n1) in enumerate(N_CHUNKS):
                            nc.tensor.matmul(
                                out=o_ps[ci][:, :n1 - n0],
                                lhsT=yq_pair(q), rhs=a_pair(ci, q),
                                perf_mode=mybir.MatmulPerfMode.DoubleRow,
                                start=False,
                                stop=(last and qi == len(qs) - 1))
                for ci, (n0, n1) in enumerate(N_CHUNKS):
                    cw = n1 - n0
                    nc.scalar.activation(
                        out=outsb[ci][:, :cw], in_=o_ps[ci][:, :cw],
                        func=mybir.ActivationFunctionType.Identity,
                        scale=1.0 / OSCALE, bias=b2s[:, 0:1])
                    nc.sync.dma_start(out=out_d[:, n0:n1],
                                      in_=outsb[ci][:, :cw])

    nc.compile()
    return nc


def _get_nc():
    global _compiled_nc
    if _compiled_nc is None:
        _compiled_nc = _build_nc()
    return _compiled_nc


def _enable_profile_hook():
    """Register the NTFF profiling hook that trn_boot skips when the image's
    antenv lacks axon_hooks (profiling only; used when GNN_PROFILE=1)."""
    try:
        import antenv
        if "antenv.axon_hooks" not in sys.modules:
            mod = types.ModuleType("antenv.axon_hooks")
            _h = [None]
            mod.set_axon_ntff_profile_hook = lambda hook: _h.__setitem__(0, hook)
            mod.get_axon_ntff_profile_hook = lambda: _h[0]
            sys.modules["antenv.axon_hooks"] = mod
            antenv.axon_hooks = mod
        from trn_agent_boot.trn_boot import _ntff_profile_via_ctypes
        hook = _ntff_profile_via_ctypes("/opt/axon/libaxon_pjrt.so")
        if hook is not None:
            sys.modules["antenv.axon_hooks"].set_axon_ntff_profile_hook(hook)
            return True
    except Exception:
        pass
    return False


def _host_prep(h, w, src, dst, W1, b1, W2, b2):
    import ml_dtypes
    import scipy.sparse as sp
    deg = np.bincount(dst, minlength=N_NODES).astype(np.float32)
    w_norm = (w[:, 0] * (ASCALE / np.maximum(deg, 1.0)[dst])).astype(np.float32)
    # pad node ids per-core to 1280 so global k-blocks align with ranks
    src_pad = (NBP * (src // NB) + src % NB).astype(np.int64)
    # AT[s_pad, d] = sum of scaled w_norm over edges (s -> d): 64*A^T
    AT = sp.coo_matrix((w_norm, (src_pad, dst)),
                       shape=(NPAD, N_NODES)).toarray()
    AT8 = AT.astype(ml_dtypes.float8_e4m3)
    hp_f = np.zeros((NPAD, D_IN), dtype=np.float32)
    for c in range(N_CORES):
        hp_f[c * NBP:c * NBP + NB] = h[c * NB:(c + 1) * NB]
    hp = hp_f.astype(ml_dtypes.float8_e4m3)
    # h8[p, k*128+f] = h[pad node 128k+p, f]
    h8 = np.ascontiguousarray(
        hp.reshape(KB, P, D_IN).transpose(1, 0, 2).reshape(P, KB * D_IN))

    # packed f16 consts: ht | W1 (hN half pre-divided by ASCALE) | W2a' |
    # W2b' | b1 | b2 -- one wide-row DMA instead of ~900 tiny descriptors
    hcat = np.zeros((P, HC_W), dtype=np.float16)
    w1c = W1.astype(np.float16)
    w1c[D_IN:] = (W1[D_IN:] / ASCALE).astype(np.float16)
    for k in range(2):
        hcat[:, HC_W1 + k * D_HID:HC_W1 + (k + 1) * D_HID] = \
            w1c[k * P:(k + 1) * P, :]
        hcat[:, HC_W2A + k * D_OUT:HC_W2A + (k + 1) * D_OUT] = (
            W2[k * P:(k + 1) * P, :] * OSCALE).astype(np.float16)
        hcat[:, HC_W2B + k * D_OUT:HC_W2B + (k + 1) * D_OUT] = (
            W2[2 * P + k * P:2 * P + (k + 1) * P, :] * YSCALE).astype(np.float16)
    hcat[:, HC_B1:HC_B1 + 2] = b1.reshape(2, P).T.astype(np.float16)
    hcat[:D_OUT, HC_B2] = b2.astype(np.float16)

    in_maps = []
    for c in range(N_CORES):
        sl = slice(c * NB, (c + 1) * NB)
        ATc = AT8[:, sl]
        hc = hcat.copy()
        hc[:, HC_HT:HC_HT + NB] = h[sl].T.astype(np.float16)
        im = {
            "h8": h8,
            "hcat": hc,
        }
        # chunk-major A stream: asc{ci}[g, p, (j two d)] for pair q=g*gp+j
        for ci, ((n0, n1), (gp, ng)) in enumerate(zip(N_CHUNKS, CGRP)):
            cw = n1 - n0
            ATcc = ATc[:, n0:n1]
            im[f"asc{ci}"] = np.ascontiguousarray(
                ATcc.reshape(ng, gp, 2, P, cw).transpose(0, 3, 1, 2, 4)
                .reshape(ng, P, gp * 2 * cw))
        in_maps.append(im)
    return in_maps


def kernel(h, w, src, dst, W1, b1, W2, b2):
    global LAST_EXEC_NS
    h = np.asarray(h, dtype=np.float32)
    w = np.asarray(w, dtype=np.float32)
    src = np.asarray(src)
    dst = np.asarray(dst)
    W1 = np.asarray(W1, dtype=np.float32)
    b1 = np.asarray(b1, dtype=np.float32)
    W2 = np.asarray(W2, dtype=np.float32)
    b2 = np.asarray(b2, dtype=np.float32)

    in_maps = _host_prep(h, w, src, dst, W1, b1, W2, b2)
    nc = _get_nc()
    trace = os.environ.get("GNN_PROFILE") == "1" and _enable_profile_hook()
    if trace:
        # steady-state measurement: first execution warms the device
        # (NEFF load, CC ring init); the traced run reflects the kernel
        bass_utils.run_bass_kernel_spmd(
            nc, in_maps, core_ids=list(range(N_CORES)), trace=False)
    res = bass_utils.run_bass_kernel_spmd(
        nc, in_maps, core_ids=list(range(N_CORES)), trace=trace)
    LAST_EXEC_NS = res.exec_time_ns

    out = np.concatenate(
        [res.results[c]["outT"].T for c in range(N_CORES)], axis=0)
    return out.astype(np.float32)
